# revision 1
# baseline (speedup 1.0000x reference)
"""Trainium2 Bass kernel for nn_DecoderLayer (conv-QKV attention + conv FFN).

Sharding: 8 cores = 4 batches x 2 token-halves. Each core computes the full
attention + FFN for 1024 tokens of one batch element. The 4 halo context
tokens each core's FFN conv needs (s-2, s-1, e+1, e+2) are computed on the
host (~0.4% of total FLOPs) and passed in, so all device-side tiling is a
uniform 512 wide.

All matmuls run as float32r (full PE rate). Layout choices:
  x, q, k       channel-major  [ch(part), tok(free)]
  v             token-major, ones-augmented per head ([kt, 33] lhsT slices
                -> one M=33 matmul accumulates ctx rows 0..31 + softmax
                denominator in row 32)
  scores/probs  [kt(part), q(free)] pairs of heads in one [128,1024] PSUM
                tensor so exp is a single wide ACT op
  y1            channel-major; y2/LN/residual token-major
"""

import contextlib

import numpy as np

import concourse.bass as bass
import concourse.mybir as mybir
import concourse.tile as tile
from concourse import bacc, bass_utils

F32 = mybir.dt.float32
F32R = mybir.dt.float32r
AF = mybir.ActivationFunctionType
ALU = mybir.AluOpType

B, L, D = 4, 2048, 256
H, DH, DFF = 8, 32, 1024
LN_EPS = 1e-5
HALF = L // 2          # tokens per core
NCORES = 8
INV_SQRT_H = 1.0 / np.sqrt(np.float32(H))

_cache = {}


def _bcast_ap(t, row, width, parts):
    """DRAM row -> all-partition broadcast AP."""
    a = t[row : row + 1, :width]
    return bass.AP(tensor=a.tensor, offset=a.offset, ap=[[0, parts]] + a.ap[1:])


def build_nc():
    nc = bacc.Bacc("TRN2", target_bir_lowering=False, debug=False)

    # ---- DRAM I/O (per-core) ----
    xcm = nc.dram_tensor("xcm", [D, L + 4], F32R, kind="ExternalInput")
    xres = nc.dram_tensor("xres", [HALF, D], F32, kind="ExternalInput")
    wq = nc.dram_tensor("wq", [3, D, D], F32R, kind="ExternalInput")
    wk = nc.dram_tensor("wk", [3, D, D], F32R, kind="ExternalInput")
    wv = nc.dram_tensor("wv", [3, D, D], F32R, kind="ExternalInput")
    w1 = nc.dram_tensor("w1", [3, D, DFF], F32R, kind="ExternalInput")
    w2 = nc.dram_tensor("w2", [3, DFF, D], F32R, kind="ExternalInput")
    bqv = nc.dram_tensor("bqv", [3, D], F32, kind="ExternalInput")   # bq, bk, bv
    b1d = nc.dram_tensor("b1d", [DFF], F32, kind="ExternalInput")
    b2d = nc.dram_tensor("b2d", [D], F32, kind="ExternalInput")
    gam = nc.dram_tensor("gam", [D], F32, kind="ExternalInput")
    ctxh = nc.dram_tensor("ctxh", [D, 4], F32R, kind="ExternalInput")
    out = nc.dram_tensor("out", [HALF, D], F32, kind="ExternalOutput")

    b1r = b1d.ap().rearrange("(a b) -> a b", b=1)    # [1024, 1]

    with tile.TileContext(nc) as tc:
        est = contextlib.ExitStack()
        with est:
            # ================= persistent SBUF =================
            # Pool creation order must nest LIFO; instruction emission order
            # (= DMA priority) is separate: x first, then qkv weights, then
            # FFN weights (consumed last).
            pw = est.enter_context(tc.tile_pool(name="pw", bufs=1))
            pctx_cm = est.enter_context(tc.tile_pool(name="pctx_cm", bufs=1))
            pa = est.enter_context(tc.tile_pool(name="pa_acts", bufs=1))
            pax_ctx = contextlib.ExitStack()
            pax = pax_ctx.enter_context(tc.tile_pool(name="pa_x", bufs=1))
            x_sb = []
            for it in range(2):
                t = pax.tile([128, L + 4], F32R, name=f"x{it}", tag=f"x{it}")
                # split columns so the first conv chunks start before the
                # whole 1MB row block lands
                nc.sync.dma_start(
                    t[:, 0:700], xcm.ap()[128 * it : 128 * it + 128, 0:700]
                )
                x_sb.append(t)
            for it in range(2):
                nc.sync.dma_start(
                    x_sb[it][:, 700:1400],
                    xcm.ap()[128 * it : 128 * it + 128, 700:1400],
                )
            for it in range(2):
                nc.sync.dma_start(
                    x_sb[it][:, 1400 : L + 4],
                    xcm.ap()[128 * it : 128 * it + 128, 1400 : L + 4],
                )
            wq_sb, wk_sb, wv_sb, w1_sb, w2_sb = [], [], [], [], []
            for k in range(3):
                for it in range(2):
                    t = pw.tile([128, D], F32R, name=f"wq{k}{it}", tag=f"wq{k}{it}")
                    nc.sync.dma_start(t[:], wq.ap()[k, 128 * it : 128 * it + 128, :])
                    wq_sb.append(t)
                    t = pw.tile([128, D], F32R, name=f"wk{k}{it}", tag=f"wk{k}{it}")
                    nc.sync.dma_start(t[:], wk.ap()[k, 128 * it : 128 * it + 128, :])
                    wk_sb.append(t)
                    t = pw.tile([128, D], F32R, name=f"wv{k}{it}", tag=f"wv{k}{it}")
                    nc.sync.dma_start(t[:], wv.ap()[k, 128 * it : 128 * it + 128, :])
                    wv_sb.append(t)
            for k in range(3):
                for it in range(2):
                    t = pw.tile([128, DFF], F32R, name=f"w1{k}{it}", tag=f"w1{k}{it}")
                    nc.sync.dma_start(t[:], w1.ap()[k, 128 * it : 128 * it + 128, :])
                    w1_sb.append(t)
            for k in range(3):
                for it in range(8):
                    t = pw.tile([128, D], F32R, name=f"w2{k}{it}", tag=f"w2{k}{it}")
                    nc.sync.dma_start(t[:], w2.ap()[k, 128 * it : 128 * it + 128, :])
                    w2_sb.append(t)

            def WQ(k, it):
                return wq_sb[k * 2 + it]

            def WK(k, it):
                return wk_sb[k * 2 + it]

            def WV(k, it):
                return wv_sb[k * 2 + it]

            def W1(k, it):
                return w1_sb[k * 2 + it]

            def W2(k, it):
                return w2_sb[k * 8 + it]

            # biases: channel-major per-partition [128,1] slices
            bq_sb, bk_sb, bv_bc, b1_sb = [], [], None, []
            bqv_r = bqv.ap()
            for it in range(2):
                t = pw.tile([128, 1], F32, name=f"bq{it}", tag=f"bq{it}")
                nc.gpsimd.dma_start(
                    t[:],
                    bqv_r[0, 128 * it : 128 * it + 128].rearrange("(a b) -> a b", b=1),
                )
                bq_sb.append(t)
                t = pw.tile([128, 1], F32, name=f"bk{it}", tag=f"bk{it}")
                nc.gpsimd.dma_start(
                    t[:],
                    bqv_r[1, 128 * it : 128 * it + 128].rearrange("(a b) -> a b", b=1),
                )
                bk_sb.append(t)
            bv_bc = pw.tile([128, D], F32, name="bv_bc", tag="bv_bc")
            nc.gpsimd.dma_start(bv_bc[:], _bcast_ap(bqv.ap(), 2, D, 128))
            for it in range(8):
                t = pw.tile([128, 1], F32, name=f"b1_{it}", tag=f"b1_{it}")
                nc.gpsimd.dma_start(t[:], b1r[128 * it : 128 * it + 128, :])
                b1_sb.append(t)
            b2_bc = pw.tile([128, D], F32, name="b2_bc", tag="b2_bc")
            nc.gpsimd.dma_start(
                b2_bc[:], _bcast_ap(b2d.ap().rearrange("(a b) -> b a", b=1), 0, D, 128)
            )
            gam_bc = pw.tile([128, D], F32, name="gam_bc", tag="gam_bc")
            nc.gpsimd.dma_start(
                gam_bc[:], _bcast_ap(gam.ap().rearrange("(a b) -> b a", b=1), 0, D, 128)
            )
            eps_sb = pw.tile([128, 1], F32, name="eps_sb", tag="eps_sb")
            nc.vector.memset(eps_sb[:], LN_EPS)

            # ctx_cm: [2 tiles][128, 1028]; col j <-> token s-2+j
            ctx_cm = []
            for it in range(2):
                t = pctx_cm.tile([128, 1028], F32R, name=f"ctxcm{it}", tag=f"ctxcm{it}")
                ctx_cm.append(t)
                # halo: cols 0,1 and 1026,1027 from host
                nc.gpsimd.dma_start(t[:, 0:2], ctxh.ap()[128 * it : 128 * it + 128, 0:2])
                nc.gpsimd.dma_start(
                    t[:, 1026:1028], ctxh.ap()[128 * it : 128 * it + 128, 2:4]
                )

            # ================ phase A: QKV convs ================
            q_sb, k_sb = [], []
            for it in range(2):
                t = pa.tile([128, HALF], F32R, name=f"q{it}", tag=f"q{it}")
                q_sb.append(t)
                t = pa.tile([128, L], F32R, name=f"k{it}", tag=f"k{it}")
                k_sb.append(t)
            v_aug = pa.tile([128, 16 * 264], F32R, name="v_aug", tag="v_aug")
            # ones columns (col 32 of each head block); f32r memset is not an
            # ISA-legal value type, so copy from a broadcast f32 ones tile
            ones_sb = pw.tile([128, 1], F32, name="ones_sb", tag="ones_sb")
            nc.vector.memset(ones_sb[:], 1.0)
            va = v_aug[:]
            nc.vector.tensor_copy(
                bass.AP(tensor=va.tensor, offset=va.offset + 32,
                        ap=[va.ap[0], [264, 16], [33, 8]]),
                ones_sb[:].to_broadcast((128, 16 * 8)),
            )

            with (
                tc.tile_pool(name="pa_ps", bufs=3, space="PSUM") as paps,
                tc.tile_pool(name="pa_psv", bufs=2, space="PSUM") as papsv,
            ):
                # q conv: out tokens [s, e]; q_sb col j <-> token s+j.
                # chunk-inner order keeps consecutive matmuls on the same
                # stationary weight tile
                for ot in range(2):
                    pss = [paps.tile([128, 512], F32, name=f"ps_q{c}", tag="ps_qk")
                           for c in range(2)]
                    n = 0
                    for it in range(2):
                        for k in range(3):
                            for ch in range(2):
                                nc.tensor.matmul(
                                    pss[ch][:],
                                    WQ(k, it)[:, 128 * ot : 128 * ot + 128],
                                    x_sb[it][:, 512 * ch + k + 1 : 512 * ch + k + 513],
                                    start=(n < 2),
                                    stop=(n >= 10),
                                )
                                n += 1
                    for ch in range(2):
                        nc.vector.tensor_scalar_add(
                            q_sb[ot][:, 512 * ch : 512 * ch + 512], pss[ch][:],
                            bq_sb[ot][:],
                        )
                # k conv: k_sb col j <-> token (s-1+j) mod L. The k/v token
                # axis is a circular shift of the true axis - softmax over all
                # 2048 keys is permutation invariant, so only q vs (k,v)
                # consistency matters.
                for ot in range(2):
                    for cg in range(2):
                        pss = [paps.tile([128, 512], F32, name=f"ps_k{c}", tag="ps_qk")
                               for c in range(2)]
                        n = 0
                        for it in range(2):
                            for k in range(3):
                                for c in range(2):
                                    ch = 2 * cg + c
                                    nc.tensor.matmul(
                                        pss[c][:],
                                        WK(k, it)[:, 128 * ot : 128 * ot + 128],
                                        x_sb[it][:, 512 * ch + k : 512 * ch + k + 512],
                                        start=(n < 2),
                                        stop=(n >= 10),
                                    )
                                    n += 1
                        for c in range(2):
                            ch = 2 * cg + c
                            nc.vector.tensor_scalar_add(
                                k_sb[ot][:, 512 * ch : 512 * ch + 512], pss[c][:],
                                bk_sb[ot][:],
                            )
                # v conv: token-major, out [t(128), o(256)] per kt tile
                for kt in range(16):
                    ps = papsv.tile([128, D], F32, name="ps_v", tag="ps_v")
                    n = 0
                    for it in range(2):
                        for k in range(3):
                            nc.tensor.matmul(
                                ps[:],
                                x_sb[it][:, 128 * kt + k : 128 * kt + k + 128],
                                WV(k, it)[:],
                                start=(n == 0),
                                stop=(n == 5),
                            )
                            n += 1
                    vv = v_aug[:]
                    vout = bass.AP(
                        tensor=vv.tensor, offset=vv.offset + 264 * kt,
                        ap=[vv.ap[0], [33, 8], [1, 32]],
                    )
                    nc.vector.scalar_tensor_tensor(
                        vout, ps[:], 1.0, bv_bc[:], op0=ALU.mult, op1=ALU.add
                    )

            pax_ctx.close()

            # ====== phases B+C: attention with interleaved FFN conv1 ======
            # PSUM budget during attention: conv1 2 + scores 4 + ctx 2 = 8.
            pbc = contextlib.ExitStack()
            pcy1 = pbc.enter_context(tc.tile_pool(name="pc_y1", bufs=1))
            y1_sb = []
            for it in range(8):
                t = pcy1.tile([128, 1026], F32R, name=f"y1_{it}", tag=f"y1_{it}")
                y1_sb.append(t)
            pcps = pbc.enter_context(tc.tile_pool(name="pc_ps", bufs=1, space="PSUM"))

            def conv1_region(c0, w, ots=range(8)):
                # y1 col j <-> token s-1+j; needs ctx_cm cols [j+k]
                for ot in ots:
                    ps = pcps.tile([128, 512], F32, name="ps_y1", tag="ps_y1")
                    n = 0
                    for it in range(2):
                        for k in range(3):
                            nc.tensor.matmul(
                                ps[:, 0:w],
                                W1(k, it)[:, 128 * ot : 128 * ot + 128],
                                ctx_cm[it][:, c0 + k : c0 + k + w],
                                start=(n == 0), stop=(n == 5),
                            )
                            n += 1
                    nc.vector.tensor_scalar(
                        y1_sb[ot][:, c0 : c0 + w], ps[:, 0:w],
                        b1_sb[ot][:], 0.0, op0=ALU.add, op1=ALU.max,
                    )

            patt = contextlib.ExitStack()
            pbsc = patt.enter_context(tc.tile_pool(name="pb_sc", bufs=2, space="PSUM"))
            pbctx = patt.enter_context(tc.tile_pool(name="pb_ctx", bufs=3, space="PSUM"))
            pbp = patt.enter_context(tc.tile_pool(name="pb_probs", bufs=6))
            pbden = patt.enter_context(tc.tile_pool(name="pb_den", bufs=2))
            pbdram = patt.enter_context(tc.tile_pool(name="pb_dram", bufs=2, space="DRAM"))

            for ch in range(2):
                for hp in range(4):
                    h0, h1 = 2 * hp, 2 * hp + 1
                    ki = h0 // 4
                    p0, p1 = 32 * (h0 % 4), 32 * (h1 % 4)
                    cA = pbctx.tile([33, 512], F32, name="ctxA", tag="ctx")
                    cB = pbctx.tile([33, 512], F32, name="ctxB", tag="ctx")
                    for kt in range(16):
                        sc = pbsc.tile([128, 1024], F32, name="sc", tag="sc")
                        nc.tensor.matmul(
                            sc[:, 0:512],
                            k_sb[ki][p0 : p0 + 32, 128 * kt : 128 * kt + 128],
                            q_sb[ki][p0 : p0 + 32, 512 * ch : 512 * ch + 512],
                            start=True, stop=True, tile_position=(p0, 0),
                        )
                        nc.tensor.matmul(
                            sc[:, 512:1024],
                            k_sb[ki][p1 : p1 + 32, 128 * kt : 128 * kt + 128],
                            q_sb[ki][p1 : p1 + 32, 512 * ch : 512 * ch + 512],
                            start=True, stop=True, tile_position=(p1, 0),
                        )
                        pr = pbp.tile([128, 1024], F32R, name="pr", tag="pr")
                        nc.scalar.activation(
                            pr[:], sc[:], AF.Exp, bias=0.0, scale=float(INV_SQRT_H)
                        )
                        nc.tensor.matmul(
                            cA[:],
                            v_aug[:, 264 * kt + 33 * h0 : 264 * kt + 33 * h0 + 33],
                            pr[:, 0:512],
                            start=(kt == 0), stop=(kt == 15),
                        )
                        nc.tensor.matmul(
                            cB[:],
                            v_aug[:, 264 * kt + 33 * h1 : 264 * kt + 33 * h1 + 33],
                            pr[:, 512:1024],
                            start=(kt == 0), stop=(kt == 15),
                        )
                    # denominators -> reciprocal -> DRAM roundtrip bcast
                    dp = pbden.tile([64, 512], F32, name="dp", tag="dp")
                    nc.vector.memset(dp[:], 1.0)
                    nc.vector.tensor_copy(dp[0:1, :], cA[32:33, :])
                    nc.vector.tensor_copy(dp[32:33, :], cB[32:33, :])
                    rc = pbden.tile([64, 512], F32, name="rc", tag="rc")
                    nc.vector.reciprocal_approx_fast(rc[:], dp[:])
                    dr = pbdram.tile([2, 512], F32, name="dr", tag="dr")
                    rca = rc[0:1, :]
                    nc.sync.dma_start(
                        dr[:],
                        bass.AP(tensor=rca.tensor, offset=rca.offset,
                                ap=[[32 * rca.ap[0][0], 2]] + rca.ap[1:]),
                    )
                    rb = pbden.tile([64, 512], F32, name="rb", tag="rb")
                    for j in range(2):
                        da = dr[j : j + 1, :]
                        nc.sync.dma_start(
                            rb[32 * j : 32 * j + 32, :],
                            bass.AP(tensor=da.tensor, offset=da.offset,
                                    ap=[[0, 32]] + da.ap[1:]),
                        )
                    for j, (cc, hh) in enumerate(((cA, h0), (cB, h1))):
                        nc.vector.tensor_mul(
                            ctx_cm[hh // 4][
                                32 * (hh % 4) : 32 * (hh % 4) + 32,
                                2 + 512 * ch : 2 + 512 * ch + 512,
                            ],
                            cc[0:32, :],
                            rb[32 * j : 32 * j + 32, :],
                        )
                    # fine-grained filler: after each ch=1 block, run two
                    # conv1 column-groups of the ch=0 region so the PE has
                    # independent work across the block-tail stall
                    if ch == 1:
                        conv1_region(0, 512, ots=range(2 * hp, 2 * hp + 2))
            conv1_region(512, 512)
            conv1_region(1024, 2)
            patt.close()

            # ============== conv2 + LN + residual ==============
            pc2 = contextlib.ExitStack()
            pcps2 = pc2.enter_context(tc.tile_pool(name="pc_ps2", bufs=2, space="PSUM"))
            pcsb = pc2.enter_context(tc.tile_pool(name="pc_sb", bufs=3))
            pcln = pc2.enter_context(tc.tile_pool(name="pc_ln", bufs=4))
            pcxr = pc2.enter_context(tc.tile_pool(name="pc_xr", bufs=3))
            for tt in range(8):
                ps = pcps2.tile([128, D], F32, name="ps_y2", tag="ps_y2")
                n = 0
                for it in range(8):
                    for k in range(3):
                        nc.tensor.matmul(
                            ps[:],
                            y1_sb[it][:, 128 * tt + k : 128 * tt + k + 128],
                            W2(k, it)[:],
                            start=(n == 0), stop=(n == 23),
                        )
                        n += 1
                y2 = pcsb.tile([128, D], F32, name="y2", tag="y2")
                nc.vector.scalar_tensor_tensor(
                    y2[:], ps[:], 1.0, b2_bc[:], op0=ALU.mult, op1=ALU.add
                )
                st = pcln.tile([128, 6], F32, name="st", tag="st")
                nc.vector.bn_stats(st[:], y2[:])
                mv = pcln.tile([128, 2], F32, name="mv", tag="mv")
                nc.vector.bn_aggr(mv[:], st[:])
                sd = pcln.tile([128, 1], F32, name="sd", tag="sd")
                nc.scalar.activation(sd[:], mv[:, 1:2], AF.Sqrt, bias=eps_sb[:])
                rs = pcln.tile([128, 1], F32, name="rs", tag="rs")
                nc.vector.reciprocal(rs[:], sd[:])
                yn = pcsb.tile([128, D], F32, name="yn", tag="yn")
                nc.vector.scalar_tensor_tensor(
                    yn[:], y2[:], mv[:, 0:1], rs[:].to_broadcast((128, D)),
                    op0=ALU.subtract, op1=ALU.mult,
                )
                xr = pcxr.tile([128, D], F32, name="xr", tag="xr")
                nc.gpsimd.dma_start(
                    xr[:], xres.ap()[128 * tt : 128 * tt + 128, :]
                )
                yg = pcsb.tile([128, D], F32, name="yg", tag="yg")
                nc.vector.tensor_mul(yg[:], yn[:], gam_bc[:])
                yo = pcsb.tile([128, D], F32, name="yo", tag="yo")
                nc.vector.tensor_add(yo[:], yg[:], xr[:])
                nc.gpsimd.dma_start(
                    out.ap()[128 * tt : 128 * tt + 128, :], yo[:]
                )
            pc2.close()
            pbc.close()

    nc.compile()
    return nc


def _host_attn_tokens(xb, toks, Wq, bq, Wk, bk, Wv, bv):
    """Attention output (pre-FFN context) rows for the given tokens, numpy."""
    k_full = np.zeros((L, D), np.float32)
    v_full = np.zeros((L, D), np.float32)
    for k in range(3):
        xs = np.roll(xb, 1 - k, axis=0)  # xs[t] = xb[(t + k - 1) % L]
        k_full += xs @ Wk[:, :, k].T
        v_full += xs @ Wv[:, :, k].T
    k_full += bk
    v_full += bv
    q8 = np.zeros((len(toks), D), np.float32)
    for k in range(3):
        idx = (toks + k - 1) % L
        q8 += xb[idx] @ Wq[:, :, k].T
    q8 += bq

    ctx8 = np.zeros((len(toks), D), np.float32)
    for h in range(H):
        sl = slice(32 * h, 32 * h + 32)
        s = (q8[:, sl] @ k_full[:, sl].T) * INV_SQRT_H  # [len, L]
        s = s - s.max(axis=1, keepdims=True)
        e = np.exp(s)
        p = e / e.sum(axis=1, keepdims=True)
        ctx8[:, sl] = p @ v_full[:, sl]
    return ctx8


def kernel(x, Wq, bq, Wk, bk, Wv, bv, W1, b1, W2, b2, gamma, beta):
    x = np.asarray(x, np.float32)
    Wq, Wk, Wv = (np.asarray(a, np.float32) for a in (Wq, Wk, Wv))
    W1, W2 = np.asarray(W1, np.float32), np.asarray(W2, np.float32)
    bq, bk, bv = (np.asarray(a, np.float32) for a in (bq, bk, bv))
    b1, b2 = np.asarray(b1, np.float32), np.asarray(b2, np.float32)
    gamma, beta = np.asarray(gamma, np.float32), np.asarray(beta, np.float32)

    if "nc" not in _cache:
        _cache["nc"] = build_nc()
    nc = _cache["nc"]

    # host-side weight transposes: [k][i][o]
    wq_t = np.ascontiguousarray(Wq.transpose(2, 1, 0))
    wk_t = np.ascontiguousarray(Wk.transpose(2, 1, 0))
    wv_t = np.ascontiguousarray(Wv.transpose(2, 1, 0))
    w1_t = np.ascontiguousarray(W1.transpose(2, 1, 0))
    w2_t = np.ascontiguousarray(W2.transpose(2, 1, 0))
    bqv = np.stack([bq, bk, bv])

    # halo ctx (host, fp32): per batch, the 8 boundary tokens both halves need
    all_toks = np.array([2046, 2047, 1024, 1025, 1022, 1023, 0, 1])
    ctx8_by_b = [
        _host_attn_tokens(x[b], all_toks, Wq, bq, Wk, bk, Wv, bv)
        for b in range(B)
    ]
    in_maps = []
    for c in range(NCORES):
        b, half = c // 2, c % 2
        s = half * HALF
        xb = x[b]
        sel = [0, 1, 2, 3] if half == 0 else [4, 5, 6, 7]
        ctx4 = ctx8_by_b[b][sel]  # rows: s-2, s-1, e+1, e+2
        ctx4_cm = np.ascontiguousarray(ctx4.T)  # [256, 4]

        xbT = np.ascontiguousarray(xb.T)  # [256, 2048]
        # xcm col j <-> token (s - 2 + j) mod L, j in [0, 2052)
        idx = (np.arange(L + 4) + s - 2) % L
        xcm = np.ascontiguousarray(xbT[:, idx])
        xres = xb[s : s + HALF] + beta[None, :]

        in_maps.append({
            "xcm": xcm,
            "xres": np.ascontiguousarray(xres),
            "wq": wq_t, "wk": wk_t, "wv": wv_t, "w1": w1_t, "w2": w2_t,
            "bqv": bqv, "b1d": b1, "b2d": b2, "gam": gamma,
            "ctxh": ctx4_cm,
        })

    res = bass_utils.run_bass_kernel_spmd(nc, in_maps, core_ids=list(range(NCORES)))
    y = np.empty((B, L, D), np.float32)
    for c in range(NCORES):
        b, half = c // 2, c % 2
        y[b, half * HALF : (half + 1) * HALF] = res.results[c]["out"]
    return y



# revision 2
# speedup vs baseline: 1.0408x; 1.0408x over previous
"""Trainium2 Bass kernel for nn_DecoderLayer — bf16 restructure.

Sharding: 8 cores = 4 batches x 2 token-halves (1024 q tokens/core, full
2048-token K/V). All matmuls bf16 (fp32 PSUM accumulate). Scalar-engine exp
is the pacing engine (~151us); everything else (QKV convs, FFN convs, LN)
is interleaved under it as PE/DVE filler work.

Attention per block (ch: q-half of 512, hp: head pair):
  scores  [kt(part), q(free)] pairs row-packed (tile_position p0/p1)
  exp     A/B alternation: A=[128,2048] (2 kt tiles, one ACT instr),
          B=[128,1024] -> bf16 probs in SBUF
  ctx     pair col-packed M=33 at tile_position (0,0)/(0,64), one PSUM bank,
          ones-column augmentation accumulates softmax denominators
  den     reciprocal + DRAM-roundtrip partition broadcast
LayerNorm rsqrt runs on DVE (Quake InvSqrt + 2 Newton steps) so the ACT
table stays on the exp set for the whole kernel.
"""

import contextlib

import ml_dtypes
import numpy as np

import concourse.bass as bass
import concourse.mybir as mybir
import concourse.tile as tile
from concourse import bacc, bass_utils

F32 = mybir.dt.float32
BF16 = mybir.dt.bfloat16
I32 = mybir.dt.int32
AF = mybir.ActivationFunctionType
ALU = mybir.AluOpType

B, L, D = 4, 2048, 256
H, DH, DFF = 8, 32, 1024
LN_EPS = 1e-5
HALF = L // 2
NCORES = 8
INV_SQRT_H = 1.0 / np.sqrt(np.float32(H))

# fallback switches (flipped if hw rejects the packed/int paths)
CTX_PACKED = True      # ctx pair col-packed M=33 at (0,0)/(0,64)
QUAKE_RSQRT = True     # LN rsqrt on DVE via int bit tricks

_cache = {}


def _bcast_ap(t, row, width, parts):
    a = t[row : row + 1, :width]
    return bass.AP(tensor=a.tensor, offset=a.offset, ap=[[0, parts]] + a.ap[1:])


def build_nc():
    nc = bacc.Bacc("TRN2", target_bir_lowering=False, debug=False)

    xcm = nc.dram_tensor("xcm", [D, L + 4], BF16, kind="ExternalInput")
    xres = nc.dram_tensor("xres", [HALF, D], F32, kind="ExternalInput")
    wq = nc.dram_tensor("wq", [3, D, D], BF16, kind="ExternalInput")
    wk = nc.dram_tensor("wk", [3, D, D], BF16, kind="ExternalInput")
    wv = nc.dram_tensor("wv", [3, D, D], BF16, kind="ExternalInput")
    w1 = nc.dram_tensor("w1", [3, D, DFF], BF16, kind="ExternalInput")
    w2 = nc.dram_tensor("w2", [3, DFF, D], BF16, kind="ExternalInput")
    bqv = nc.dram_tensor("bqv", [3, D], F32, kind="ExternalInput")
    b1d = nc.dram_tensor("b1d", [DFF], F32, kind="ExternalInput")
    b2d = nc.dram_tensor("b2d", [D], F32, kind="ExternalInput")
    gam = nc.dram_tensor("gam", [D], F32, kind="ExternalInput")
    ctxh = nc.dram_tensor("ctxh", [D, 4], BF16, kind="ExternalInput")
    out = nc.dram_tensor("out", [HALF, D], F32, kind="ExternalOutput")

    b1r = b1d.ap().rearrange("(a b) -> a b", b=1)

    with tile.TileContext(nc) as tc:
        est = contextlib.ExitStack()
        with est:
            pw = est.enter_context(tc.tile_pool(name="pw", bufs=1))
            pcm = est.enter_context(tc.tile_pool(name="pcm", bufs=1))
            py1p = est.enter_context(tc.tile_pool(name="py1", bufs=1))
            pract = est.enter_context(tc.tile_pool(name="pract", bufs=1))
            pra = est.enter_context(tc.tile_pool(name="pra", bufs=3))
            prb = est.enter_context(tc.tile_pool(name="prb", bufs=3))
            pden = est.enter_context(tc.tile_pool(name="pden", bufs=2))
            pdram = est.enter_context(
                tc.tile_pool(name="pdram", bufs=2, space="DRAM")
            )
            pout = est.enter_context(tc.tile_pool(name="pout", bufs=3))

            # ---- ACT exp-table preload (overlaps input DMA) ----
            dz = pw.tile([128, 1], F32, name="dz", tag="dz")
            nc.vector.memset(dz[:], 0.0)
            de = pw.tile([128, 1], F32, name="de", tag="de")
            nc.scalar.activation(de[:], dz[:], AF.Exp, bias=0.0, scale=1.0)

            # ---- input DMAs (sync queue: priority order) ----
            x_sb = []
            for it in range(2):
                t = pract.tile([128, L + 4], BF16, name=f"x{it}", tag=f"x{it}")
                nc.sync.dma_start(
                    t[:, 0:704], xcm.ap()[128 * it : 128 * it + 128, 0:704]
                )
                x_sb.append(t)
            # coalesced weight tiles: one DMA per tensor (issue-rate bound
            # queue: 46 small DMAs cost ~28us of issue time)
            def _wload(name, dram, nin, width):
                t = pw.tile([128, nin * 3 * width], BF16, name=name, tag=name)
                dst = t[:].rearrange("p (k i o) -> p k i o", k=3, i=nin)
                src = dram.ap().rearrange("k (i p) o -> p k i o", i=nin)
                nc.sync.dma_start(dst, src)
                return t

            wq_all = _wload("wq_all", wq, 2, D)
            wk_all = _wload("wk_all", wk, 2, D)
            wv_all = _wload("wv_all", wv, 2, D)

            def WQ(k, it):
                o = D * (2 * k + it)
                return wq_all[:, o : o + D]

            def WK(k, it):
                o = D * (2 * k + it)
                return wk_all[:, o : o + D]

            def WV(k, it):
                o = D * (2 * k + it)
                return wv_all[:, o : o + D]

            for it in range(2):
                nc.sync.dma_start(
                    x_sb[it][:, 704 : L + 4],
                    xcm.ap()[128 * it : 128 * it + 128, 704 : L + 4],
                )
            w1_all = _wload("w1_all", w1, 2, DFF)
            w2_all = _wload("w2_all", w2, 8, D)

            def W1(k, it):
                o = DFF * (2 * k + it)
                return w1_all[:, o : o + DFF]

            def W2(k, j):
                o = D * (8 * k + j)
                return w2_all[:, o : o + D]

            # ---- small/bias DMAs (gpsimd queue) ----
            bqv_r = bqv.ap()
            bq_sb, bk_sb, b1_sb = [], [], []
            for it in range(2):
                t = pw.tile([128, 1], F32, name=f"bq{it}", tag=f"bq{it}")
                nc.gpsimd.dma_start(
                    t[:],
                    bqv_r[0, 128 * it : 128 * it + 128].rearrange("(a b) -> a b", b=1),
                )
                bq_sb.append(t)
                t = pw.tile([128, 1], F32, name=f"bk{it}", tag=f"bk{it}")
                nc.gpsimd.dma_start(
                    t[:],
                    bqv_r[1, 128 * it : 128 * it + 128].rearrange("(a b) -> a b", b=1),
                )
                bk_sb.append(t)
            bv_bc = pw.tile([128, D], F32, name="bv_bc", tag="bv_bc")
            nc.gpsimd.dma_start(bv_bc[:], _bcast_ap(bqv.ap(), 2, D, 128))
            for j in range(8):
                t = pw.tile([128, 1], F32, name=f"b1_{j}", tag=f"b1_{j}")
                nc.gpsimd.dma_start(t[:], b1r[128 * j : 128 * j + 128, :])
                b1_sb.append(t)
            b2_bc = pw.tile([128, D], F32, name="b2_bc", tag="b2_bc")
            nc.gpsimd.dma_start(
                b2_bc[:], _bcast_ap(b2d.ap().rearrange("(a b) -> b a", b=1), 0, D, 128)
            )
            gam_bc = pw.tile([128, D], F32, name="gam_bc", tag="gam_bc")
            nc.gpsimd.dma_start(
                gam_bc[:], _bcast_ap(gam.ap().rearrange("(a b) -> b a", b=1), 0, D, 128)
            )

            # ---- activations / persistent intermediates ----
            q_sb = [
                pract.tile([128, HALF], BF16, name=f"q{ot}", tag=f"q{ot}")
                for ot in range(2)
            ]
            k_sb = [
                pract.tile([128, L], BF16, name=f"k{ot}", tag=f"k{ot}")
                for ot in range(2)
            ]
            v_aug = pract.tile([128, 16 * 264], BF16, name="v_aug", tag="v_aug")
            ones_sb = pw.tile([128, 1], BF16, name="ones_sb", tag="ones_sb")
            nc.vector.memset(ones_sb[:], 1.0)
            ones_row = pw.tile([1, 32], F32, name="ones_row", tag="ones_row")
            nc.vector.memset(ones_row[:], 1.0)
            va = v_aug[:]
            nc.vector.tensor_copy(
                bass.AP(tensor=va.tensor, offset=va.offset + 32,
                        ap=[va.ap[0], [264, 16], [33, 8]]),
                ones_sb[:].to_broadcast((128, 16 * 8)),
            )
            ctx_cm = []
            for it in range(2):
                t = pcm.tile([128, 1028], BF16, name=f"ctxcm{it}", tag=f"ctxcm{it}")
                ctx_cm.append(t)
                nc.gpsimd.dma_start(t[:, 0:2], ctxh.ap()[128 * it : 128 * it + 128, 0:2])
                nc.gpsimd.dma_start(
                    t[:, 1026:1028], ctxh.ap()[128 * it : 128 * it + 128, 2:4]
                )
            y1 = [
                py1p.tile([128, 1026], BF16, name=f"y1_{j}", tag=f"y1_{j}")
                for j in range(8)
            ]

            # ================= filler units =================
            def unit_q(pool, ot, c):
                ps = pool.tile([128, 512], F32, name="cv_q", tag="cv")
                n = 0
                for it in range(2):
                    for k in range(3):
                        nc.tensor.matmul(
                            ps[:],
                            WQ(k, it)[:, 128 * ot : 128 * ot + 128],
                            x_sb[it][:, 512 * c + k + 1 : 512 * c + k + 513],
                            start=(n == 0), stop=(n == 5),
                        )
                        n += 1
                nc.vector.tensor_scalar_add(
                    q_sb[ot][:, 512 * c : 512 * c + 512], ps[:], bq_sb[ot][:]
                )

            def unit_k(pool, ot, cg):
                ps = pool.tile([128, 512], F32, name="cv_k", tag="cv")
                n = 0
                for it in range(2):
                    for k in range(3):
                        nc.tensor.matmul(
                            ps[:],
                            WK(k, it)[:, 128 * ot : 128 * ot + 128],
                            x_sb[it][:, 512 * cg + k : 512 * cg + k + 512],
                            start=(n == 0), stop=(n == 5),
                        )
                        n += 1
                nc.vector.tensor_scalar_add(
                    k_sb[ot][:, 512 * cg : 512 * cg + 512], ps[:], bk_sb[ot][:]
                )

            def unit_v(pool, kt, hf):
                ps = pool.tile([128, 512], F32, name="cv_v", tag="cv")
                n = 0
                for it in range(2):
                    for k in range(3):
                        nc.tensor.matmul(
                            ps[:, 0:128],
                            x_sb[it][:, 128 * kt + k : 128 * kt + k + 128],
                            WV(k, it)[:, 128 * hf : 128 * hf + 128],
                            start=(n == 0), stop=(n == 5),
                        )
                        n += 1
                vv = v_aug[:]
                vout = bass.AP(
                    tensor=vv.tensor, offset=vv.offset + 264 * kt + 132 * hf,
                    ap=[vv.ap[0], [33, 4], [1, 32]],
                )
                nc.vector.scalar_tensor_tensor(
                    vout, ps[:, 0:128], 1.0, bv_bc[:, 128 * hf : 128 * hf + 128],
                    op0=ALU.mult, op1=ALU.add,
                )

            def unit_c1(pool, ot, c0, w):
                ps = pool.tile([128, 512], F32, name="cv_c1", tag="cv")
                n = 0
                for it in range(2):
                    for k in range(3):
                        nc.tensor.matmul(
                            ps[:, 0:w],
                            W1(k, it)[:, 128 * ot : 128 * ot + 128],
                            ctx_cm[it][:, c0 + k : c0 + k + w],
                            start=(n == 0), stop=(n == 5),
                        )
                        n += 1
                nc.vector.tensor_scalar(
                    y1[ot][:, c0 : c0 + w], ps[:, 0:w], b1_sb[ot][:], 0.0,
                    op0=ALU.add, op1=ALU.max,
                )

            eps_sb = pw.tile([128, 1], F32, name="eps_sb", tag="eps_sb")
            nc.vector.memset(eps_sb[:], LN_EPS)

            def unit_c2(pool, tt):
                ps = pool.tile([128, 512], F32, name="cv_c2", tag="cv")
                n = 0
                for j in range(8):
                    for k in range(3):
                        nc.tensor.matmul(
                            ps[:, 0:D],
                            y1[j][:, 128 * tt + k : 128 * tt + k + 128],
                            W2(k, j),
                            start=(n == 0), stop=(n == 23),
                        )
                        n += 1
                y2 = pout.tile([128, D], F32, name="y2", tag="y2")
                nc.vector.scalar_tensor_tensor(
                    y2[:], ps[:, 0:D], 1.0, b2_bc[:], op0=ALU.mult, op1=ALU.add
                )
                st = pout.tile([128, 6], F32, name="st", tag="st")
                nc.vector.bn_stats(st[:], y2[:])
                mv = pout.tile([128, 2], F32, name="mv", tag="mv")
                nc.vector.bn_aggr(mv[:], st[:])
                if QUAKE_RSQRT:
                    ve = pout.tile([128, 1], F32, name="ve", tag="ve")
                    nc.vector.tensor_scalar_add(ve[:], mv[:, 1:2], LN_EPS)
                    i1 = pout.tile([128, 1], I32, name="i1", tag="i1")
                    nc.vector.tensor_scalar(
                        i1[:], ve[:].bitcast(I32), 1, -1,
                        op0=ALU.logical_shift_right, op1=ALU.bitwise_xor,
                    )
                    y0i = pout.tile([128, 1], I32, name="y0i", tag="y0i")
                    nc.vector.tensor_scalar_add(y0i[:], i1[:], 0x5F3759DF + 1)
                    y0 = y0i[:].bitcast(F32)
                    t1 = pout.tile([128, 1], F32, name="t1", tag="t1")
                    nc.vector.tensor_mul(t1[:], y0, y0)
                    t1b = pout.tile([128, 1], F32, name="t1b", tag="t1b")
                    nc.vector.tensor_mul(t1b[:], t1[:], ve[:])
                    t1c = pout.tile([128, 1], F32, name="t1c", tag="t1c")
                    nc.vector.tensor_scalar(
                        t1c[:], t1b[:], -0.5, 1.5, op0=ALU.mult, op1=ALU.add
                    )
                    yr = pout.tile([128, 1], F32, name="yr", tag="yr")
                    nc.vector.tensor_mul(yr[:], y0, t1c[:])
                    t2 = pout.tile([128, 1], F32, name="t2", tag="t2")
                    nc.vector.tensor_mul(t2[:], yr[:], yr[:])
                    t2b = pout.tile([128, 1], F32, name="t2b", tag="t2b")
                    nc.vector.tensor_mul(t2b[:], t2[:], ve[:])
                    t2c = pout.tile([128, 1], F32, name="t2c", tag="t2c")
                    nc.vector.tensor_scalar(
                        t2c[:], t2b[:], -0.5, 1.5, op0=ALU.mult, op1=ALU.add
                    )
                    rs = pout.tile([128, 1], F32, name="rs", tag="rs")
                    nc.vector.tensor_mul(rs[:], yr[:], t2c[:])
                else:
                    sd = pout.tile([128, 1], F32, name="sd", tag="sd")
                    nc.scalar.activation(sd[:], mv[:, 1:2], AF.Sqrt, bias=eps_sb[:])
                    rs = pout.tile([128, 1], F32, name="rs", tag="rs")
                    nc.vector.reciprocal(rs[:], sd[:])
                yn = pout.tile([128, D], F32, name="yn", tag="yn")
                nc.vector.scalar_tensor_tensor(
                    yn[:], y2[:], mv[:, 0:1], rs[:].to_broadcast((128, D)),
                    op0=ALU.subtract, op1=ALU.mult,
                )
                xr = pout.tile([128, D], F32, name="xr", tag="xr")
                nc.sync.dma_start(xr[:], xres.ap()[128 * tt : 128 * tt + 128, :])
                yg = pout.tile([128, D], F32, name="yg", tag="yg")
                nc.vector.tensor_mul(yg[:], yn[:], gam_bc[:])
                yo = pout.tile([128, D], F32, name="yo", tag="yo")
                nc.vector.tensor_add(yo[:], yg[:], xr[:])
                nc.sync.dma_start(out.ap()[128 * tt : 128 * tt + 128, :], yo[:])

            def unit_kw(pool, ot, c0, w):
                # narrow k-conv chunk (cols c0:c0+w)
                ps = pool.tile([128, 512], F32, name="cv_kw", tag="cv")
                n = 0
                for it in range(2):
                    for k in range(3):
                        nc.tensor.matmul(
                            ps[:, 0:w],
                            WK(k, it)[:, 128 * ot : 128 * ot + 128],
                            x_sb[it][:, c0 + k : c0 + k + w],
                            start=(n == 0), stop=(n == 5),
                        )
                        n += 1
                nc.vector.tensor_scalar_add(
                    k_sb[ot][:, c0 : c0 + w], ps[:, 0:w], bk_sb[ot][:]
                )

            # ================= pre-phase (minimal: q chunk + k kt0-1) ======
            with tc.tile_pool(name="ppre", bufs=2, space="PSUM") as ppre:
                unit_q(ppre, 0, 0)
                unit_kw(ppre, 0, 0, 260)

            # ================= attention =================
            # filler schedule: {(block, group): [closure, ...]}
            sched = {}

            def S(bi, gi, fn, *args):
                sched.setdefault((bi, gi), []).append((fn, args))

            # block0: rest of k(ot0) + v heads 0-3
            S(0, 0, unit_v, 0, 0); S(0, 0, unit_v, 1, 0)
            S(0, 0, unit_kw, 0, 260, 252)
            S(0, 1, unit_v, 2, 0); S(0, 1, unit_k, 0, 1)
            S(0, 2, unit_v, 3, 0); S(0, 2, unit_v, 4, 0)
            S(0, 3, unit_v, 5, 0); S(0, 3, unit_k, 0, 2)
            S(0, 4, unit_v, 6, 0); S(0, 4, unit_v, 7, 0)
            S(0, 5, unit_v, 8, 0)
            S(0, 6, unit_v, 9, 0); S(0, 6, unit_v, 10, 0)
            S(0, 6, unit_k, 0, 3)
            S(0, 7, unit_v, 11, 0)
            S(0, 8, unit_v, 12, 0); S(0, 8, unit_v, 13, 0)
            S(0, 9, unit_v, 14, 0); S(0, 9, unit_v, 15, 0)
            # block1 (odd): k(ot1) + q(ot1 c0)
            S(1, 0, unit_k, 1, 0); S(1, 2, unit_k, 1, 1)
            S(1, 4, unit_k, 1, 2); S(1, 6, unit_k, 1, 3)
            S(1, 8, unit_q, 1, 0)
            # block2 (even): v heads 4-7 (kt->group map as block0)
            S(2, 0, unit_v, 0, 1); S(2, 0, unit_v, 1, 1)
            S(2, 1, unit_v, 2, 1)
            S(2, 2, unit_v, 3, 1); S(2, 2, unit_v, 4, 1)
            S(2, 3, unit_v, 5, 1)
            S(2, 4, unit_v, 6, 1); S(2, 4, unit_v, 7, 1)
            S(2, 5, unit_v, 8, 1)
            S(2, 6, unit_v, 9, 1); S(2, 6, unit_v, 10, 1)
            S(2, 7, unit_v, 11, 1)
            S(2, 8, unit_v, 12, 1); S(2, 8, unit_v, 13, 1)
            S(2, 9, unit_v, 14, 1); S(2, 9, unit_v, 15, 1)
            # block3 (odd): q chunks for ch1 + PE keep-warm (HAM cools in
            # this filler-light block, slowing the following blocks)
            def unit_warm(pool):
                ps = pool.tile([128, 512], F32, name="cv_w", tag="cv")
                for r in range(4):
                    nc.tensor.matmul(ps[:, 0:64], x_sb[0][:, 0:128],
                                     WQ(0, 0)[:, 0:64],
                                     start=(r == 0), stop=(r == 3))

            S(3, 0, unit_q, 0, 1); S(3, 2, unit_q, 1, 1)
            for g in (4, 5, 6, 7, 8, 9):
                S(3, g, unit_warm)
            # blocks 4-5: conv1 R0 spread; block6-7: conv2 tt0-2
            S(4, 0, unit_c1, 0, 0, 512); S(4, 3, unit_c1, 1, 0, 512)
            S(4, 6, unit_c1, 2, 0, 512); S(4, 9, unit_c1, 3, 0, 512)
            S(5, 0, unit_c1, 4, 0, 512); S(5, 3, unit_c1, 5, 0, 512)
            S(5, 6, unit_c1, 6, 0, 512); S(5, 9, unit_c1, 7, 0, 512)
            S(6, 0, unit_c2, 0); S(6, 5, unit_c2, 1)
            S(7, 0, unit_c2, 2)

            GROUPS_EVEN = [
                ((0, 1), "A"), ((2,), "B"), ((3, 4), "A"), ((5,), "B"),
                ((6, 7), "A"), ((8,), "B"), ((9, 10), "A"), ((11,), "B"),
                ((12, 13), "A"), ((14,), "B"), ((15,), "A1"),
            ]
            GROUPS_ODD = [
                ((0,), "B"), ((1, 2), "A"), ((3,), "B"), ((4, 5), "A"),
                ((6,), "B"), ((7, 8), "A"), ((9,), "B"), ((10, 11), "A"),
                ((12,), "B"), ((13, 14), "A"), ((15,), "B"),
            ]

            patt = contextlib.ExitStack()
            ppA = patt.enter_context(tc.tile_pool(name="ppA", bufs=1, space="PSUM"))
            ppB = patt.enter_context(tc.tile_pool(name="ppB", bufs=1, space="PSUM"))
            ppctx = patt.enter_context(tc.tile_pool(name="ppctx", bufs=1, space="PSUM"))
            ppcv = patt.enter_context(tc.tile_pool(name="ppcv", bufs=1, space="PSUM"))

            for bi in range(8):
                GROUPS = GROUPS_EVEN if bi % 2 == 0 else GROUPS_ODD
                ch, hp = bi // 4, bi % 4
                ki = hp // 2
                p0 = 64 * (hp % 2)
                p1 = p0 + 32
                h0, h1 = 2 * hp, 2 * hp + 1
                cbank = ppctx.tile([128, 512], F32, name="cbank", tag="ctx")
                pending = []  # (kt, pr, off)

                def emit_ctx(ent):
                    kt, pr, off = ent
                    if CTX_PACKED:
                        nc.tensor.matmul(
                            cbank[0:33, :],
                            v_aug[:, 264 * kt + 33 * h0 : 264 * kt + 33 * h0 + 33],
                            pr[:, off : off + 512],
                            start=(kt == 0), stop=(kt == 15),
                            tile_position=(0, 0),
                        )
                        nc.tensor.matmul(
                            cbank[64:97, :],
                            v_aug[:, 264 * kt + 33 * h1 : 264 * kt + 33 * h1 + 33],
                            pr[:, off + 512 : off + 1024],
                            start=(kt == 0), stop=(kt == 15),
                            tile_position=(0, 64),
                        )
                    else:
                        nc.tensor.matmul(
                            cbank[0:33, :],
                            v_aug[:, 264 * kt + 33 * h0 : 264 * kt + 33 * h0 + 33],
                            pr[:, off : off + 512],
                            start=(kt == 0), stop=(kt == 15),
                        )
                        nc.tensor.matmul(
                            cbank[64:97, :],
                            v_aug[:, 264 * kt + 33 * h1 : 264 * kt + 33 * h1 + 33],
                            pr[:, off + 512 : off + 1024],
                            start=(kt == 0), stop=(kt == 15),
                        )

                for gi, (kts, kind) in enumerate(GROUPS):
                    if kind == "A":
                        sc = ppA.tile([128, 2048], F32, name="scA", tag="scA")
                        pr = pra.tile([128, 2048], BF16, name="prA", tag="prA")
                        w = 2048
                    elif kind == "B":
                        sc = ppB.tile([128, 1024], F32, name="scB", tag="scB")
                        pr = prb.tile([128, 1024], BF16, name="prB", tag="prB")
                        w = 1024
                    else:  # A1: half-width reuse of the A banks
                        sc = ppA.tile([128, 2048], F32, name="scA", tag="scA")
                        pr = prb.tile([128, 1024], BF16, name="prB", tag="prB")
                        w = 1024
                    for j, kt in enumerate(kts):
                        nc.tensor.matmul(
                            sc[:, 1024 * j : 1024 * j + 512],
                            k_sb[ki][p0 : p0 + 32, 128 * kt : 128 * kt + 128],
                            q_sb[ki][p0 : p0 + 32, 512 * ch : 512 * ch + 512],
                            start=True, stop=True, tile_position=(p0, 0),
                        )
                        nc.tensor.matmul(
                            sc[:, 1024 * j + 512 : 1024 * j + 1024],
                            k_sb[ki][p1 : p1 + 32, 128 * kt : 128 * kt + 128],
                            q_sb[ki][p1 : p1 + 32, 512 * ch : 512 * ch + 512],
                            start=True, stop=True, tile_position=(p1, 0),
                        )
                    nc.scalar.activation(
                        pr[:, 0:w], sc[:, 0:w], AF.Exp,
                        bias=0.0, scale=float(INV_SQRT_H),
                    )
                    new = [(kt, pr, 1024 * j) for j, kt in enumerate(kts)]
                    # one-group ctx lag
                    for ent in pending:
                        emit_ctx(ent)
                    pending = new
                    for fn, args in sched.get((bi, gi), []):
                        fn(ppcv, *args)
                for ent in pending:
                    emit_ctx(ent)

                # ---- block end: denominators -> recip -> bcast -> ctx_cm ----
                if False:
                    pass
                else:
                    dn = pden.tile([64, 512], F32, name="dn", tag="dn")
                    nc.vector.memset(dn[:], 1.0)
                    nc.vector.tensor_copy(dn[0:1, :], cbank[32:33, :])
                    nc.vector.tensor_copy(dn[32:33, :], cbank[96:97, :])
                    rc = pden.tile([64, 512], F32, name="rc", tag="rc")
                    nc.vector.reciprocal_approx_fast(rc[:], dn[:])
                    dr = pdram.tile([2, 512], F32, name="dr", tag="dr")
                    rca = rc[0:1, :]
                    nc.sync.dma_start(
                        dr[:],
                        bass.AP(tensor=rca.tensor, offset=rca.offset,
                                ap=[[32 * rca.ap[0][0], 2]] + rca.ap[1:]),
                    )
                    rb = pden.tile([64, 512], F32, name="rb", tag="rb")
                    for j2 in range(2):
                        da = dr[j2 : j2 + 1, :]
                        nc.sync.dma_start(
                            rb[32 * j2 : 32 * j2 + 32, :],
                            bass.AP(tensor=da.tensor, offset=da.offset,
                                    ap=[[0, 32]] + da.ap[1:]),
                        )
                    for j2, (ppart, hh) in enumerate(((0, h0), (64, h1))):
                        nc.vector.tensor_mul(
                            ctx_cm[hh // 4][
                                32 * (hh % 4) : 32 * (hh % 4) + 32,
                                2 + 512 * ch : 2 + 512 * ch + 512,
                            ],
                            cbank[ppart : ppart + 32, :],
                            rb[32 * j2 : 32 * j2 + 32, :],
                        )
            patt.close()

            # ================= tail =================
            with tc.tile_pool(name="ptail", bufs=3, space="PSUM") as pt:
                for ot in range(8):
                    unit_c1(pt, ot, 512, 512)
                for ot in range(8):
                    unit_c1(pt, ot, 1024, 2)
                for tt in range(3, 8):
                    unit_c2(pt, tt)

    nc.compile()
    return nc


def _host_attn_tokens(xb, toks, Wq, bq, Wk, bk, Wv, bv):
    k_full = np.zeros((L, D), np.float32)
    v_full = np.zeros((L, D), np.float32)
    for k in range(3):
        xs = np.roll(xb, 1 - k, axis=0)
        k_full += xs @ Wk[:, :, k].T
        v_full += xs @ Wv[:, :, k].T
    k_full += bk
    v_full += bv
    q8 = np.zeros((len(toks), D), np.float32)
    for k in range(3):
        idx = (toks + k - 1) % L
        q8 += xb[idx] @ Wq[:, :, k].T
    q8 += bq

    ctx8 = np.zeros((len(toks), D), np.float32)
    for h in range(H):
        sl = slice(32 * h, 32 * h + 32)
        s = (q8[:, sl] @ k_full[:, sl].T) * INV_SQRT_H
        s = s - s.max(axis=1, keepdims=True)
        e = np.exp(s)
        p = e / e.sum(axis=1, keepdims=True)
        ctx8[:, sl] = p @ v_full[:, sl]
    return ctx8


def kernel(x, Wq, bq, Wk, bk, Wv, bv, W1, b1, W2, b2, gamma, beta):
    BF = ml_dtypes.bfloat16
    x = np.asarray(x, np.float32)
    Wq, Wk, Wv = (np.asarray(a, np.float32) for a in (Wq, Wk, Wv))
    W1, W2 = np.asarray(W1, np.float32), np.asarray(W2, np.float32)
    bq, bk, bv = (np.asarray(a, np.float32) for a in (bq, bk, bv))
    b1, b2 = np.asarray(b1, np.float32), np.asarray(b2, np.float32)
    gamma, beta = np.asarray(gamma, np.float32), np.asarray(beta, np.float32)

    if "nc" not in _cache:
        _cache["nc"] = build_nc()
    nc = _cache["nc"]

    wq_t = np.ascontiguousarray(Wq.transpose(2, 1, 0)).astype(BF)
    wk_t = np.ascontiguousarray(Wk.transpose(2, 1, 0)).astype(BF)
    wv_t = np.ascontiguousarray(Wv.transpose(2, 1, 0)).astype(BF)
    w1_t = np.ascontiguousarray(W1.transpose(2, 1, 0)).astype(BF)
    w2_t = np.ascontiguousarray(W2.transpose(2, 1, 0)).astype(BF)
    bqv = np.stack([bq, bk, bv])

    all_toks = np.array([2046, 2047, 1024, 1025, 1022, 1023, 0, 1])
    ctx8_by_b = [
        _host_attn_tokens(x[b], all_toks, Wq, bq, Wk, bk, Wv, bv)
        for b in range(B)
    ]
    in_maps = []
    for c in range(NCORES):
        b, half = c // 2, c % 2
        s = half * HALF
        xb = x[b]
        sel = [0, 1, 2, 3] if half == 0 else [4, 5, 6, 7]
        ctx4 = ctx8_by_b[b][sel]
        ctx4_cm = np.ascontiguousarray(ctx4.T).astype(BF)

        xbT = np.ascontiguousarray(xb.T)
        idx = (np.arange(L + 4) + s - 2) % L
        xcm = np.ascontiguousarray(xbT[:, idx]).astype(BF)
        xres = xb[s : s + HALF] + beta[None, :]

        in_maps.append({
            "xcm": xcm,
            "xres": np.ascontiguousarray(xres),
            "wq": wq_t, "wk": wk_t, "wv": wv_t, "w1": w1_t, "w2": w2_t,
            "bqv": bqv, "b1d": b1, "b2d": b2, "gam": gamma,
            "ctxh": ctx4_cm,
        })

    res = bass_utils.run_bass_kernel_spmd(nc, in_maps, core_ids=list(range(NCORES)))
    y = np.empty((B, L, D), np.float32)
    for c in range(NCORES):
        b, half = c // 2, c % 2
        y[b, half * HALF : (half + 1) * HALF] = res.results[c]["out"]
    return y
